# revision 1
# baseline (speedup 1.0000x reference)
"""Trainium2 Bass kernel for nn_CrossAttention (single-CLS-query cross attention).

Reference computes, per batch b:
    q = x[b,0,:] @ wq.T                  (single CLS query)
    k = x[b] @ wk.T ; v = x[b] @ wv.T
    out = softmax(q k^T / sqrt(d)) v ; y = out @ wp.T + bp

Because there is a single query token, the huge K/V projections can be
eliminated algebraically:
    scores[b,h,n] = M[b,h,:] . x[b,n,:]   with  M[b,h,:] = (SCALE*q_h) @ Wk_h
    U[b,h,:]     = sum_n attn[b,h,n] x[b,n,:]
    y[b]         = concat_h(U[b,h,:] @ Wv_h.T) @ wp.T + bp
which needs only two streaming passes over x (~2.5 GMAC total) instead of
the 155 GFLOP dense projections.

Distribution: pure data parallel over batch B=32 across 8 cores (4 batches
per core), no collectives.  Each core streams its x shard twice: once in
[C, N] layout (scores, contraction over C) and once in [N, C] layout
(weighted sum, contraction over N), since the PE can only contract over the
partition dimension.  Both layouts are prepared host-side.
"""

import numpy as np

import concourse.bass as bass
import concourse.tile as tile
from concourse import bacc, mybir
from concourse.bass_utils import run_bass_kernel_spmd

# Problem constants (hardcoded per the harness contract).
B, N, C = 32, 4096, 768
H, D = 12, 64
SCALE = D ** -0.5
NCORES = 8
BSH = B // NCORES  # batches per core

F32 = mybir.dt.float32
F32R = mybir.dt.float32r

# Phase dtype knobs.  float32r runs the PE at 1 cyc/row (vs 4 for float32)
# at reduced internal precision; float32 is the conservative choice.
C_DT = F32R  # dtype for the weighted-sum (phase C) matmuls
A_DT = F32R  # dtype for the scores (phase A) matmuls
NCHUNK = C // 128  # 6
DEBUG = False


def build_kernel():
    nc = bacc.Bacc("TRN2", target_bir_lowering=False, debug=False,
                   num_devices=NCORES)

    xT = nc.dram_tensor("xT", [BSH, C, N], A_DT, kind="ExternalInput")
    x = nc.dram_tensor("x", [BSH, N, C], C_DT, kind="ExternalInput")
    x0T = nc.dram_tensor("x0T", [C, BSH], F32, kind="ExternalInput")
    wqT = nc.dram_tensor("wqT", [C, C], F32, kind="ExternalInput")
    wk = nc.dram_tensor("wk", [C, C], F32, kind="ExternalInput")
    wvT = nc.dram_tensor("wvT", [C, C], F32, kind="ExternalInput")
    wpT = nc.dram_tensor("wpT", [C, C], F32, kind="ExternalInput")
    bp = nc.dram_tensor("bp", [1, C], F32, kind="ExternalInput")
    i12 = nc.dram_tensor("i12", [H, H], F32, kind="ExternalInput")
    y = nc.dram_tensor("y", [BSH, C], F32, kind="ExternalOutput")
    dbg = {}
    if DEBUG:
        dbg["qT"] = nc.dram_tensor("dbg_qT", [128, NCHUNK, BSH], F32,
                                   kind="ExternalOutput").ap()
        dbg["mT"] = nc.dram_tensor("dbg_mT", [128, NCHUNK, BSH, H], F32,
                                   kind="ExternalOutput").ap()
        dbg["attnT"] = nc.dram_tensor("dbg_attnT", [128, N // 128, H], F32,
                                      kind="ExternalOutput").ap()
        dbg["U"] = nc.dram_tensor("dbg_U", [H, C], F32,
                                  kind="ExternalOutput").ap()

    with tile.TileContext(nc) as tc:
        cross_attn_kernel(tc, y.ap(), xT.ap(), x.ap(), x0T.ap(), wqT.ap(),
                          wk.ap(), wvT.ap(), wpT.ap(), bp.ap(), i12.ap(), dbg)
    nc.compile()
    return nc


def cross_attn_kernel(tc, y, xT, x, x0T, wqT, wk, wvT, wpT, bp, i12, dbg={}):
    from contextlib import ExitStack
    ctx = ExitStack()
    nc = tc.nc
    with ctx:
        consts = ctx.enter_context(tc.tile_pool(name="consts", bufs=1))
        xa_pool = ctx.enter_context(tc.tile_pool(name="xa", bufs=20))
        xc_pool = ctx.enter_context(tc.tile_pool(name="xc", bufs=12))
        attn_pool = ctx.enter_context(tc.tile_pool(name="attn", bufs=2))
        small = ctx.enter_context(tc.tile_pool(name="small", bufs=2))
        ps_a = ctx.enter_context(tc.tile_pool(name="ps_a", bufs=2, space="PSUM"))
        ps_c = ctx.enter_context(tc.tile_pool(name="ps_c", bufs=1, space="PSUM"))
        ps_misc = ctx.enter_context(tc.tile_pool(name="ps_misc", bufs=2, space="PSUM"))

        # ---- constant loads ----
        # All on the scalar HWDGE queue so the sync queue starts streaming
        # x tiles immediately; wvT/wpT are deferred until P4 needs them.
        def load_w(ap_dram, name):
            t = consts.tile([128, NCHUNK, C], F32, tag=name)
            nc.scalar.dma_start(out=t, in_=ap_dram.rearrange("(a p) o -> p a o", p=128))
            return t

        wqT_sb = load_w(wqT, "wqT_sb")
        wk_sb = load_w(wk, "wk_sb")
        x0T_sb = consts.tile([128, NCHUNK, BSH], F32)
        nc.scalar.dma_start(out=x0T_sb, in_=x0T.rearrange("(a p) b -> p a b", p=128))
        i12_sb = consts.tile([H, H], F32)
        nc.scalar.dma_start(out=i12_sb, in_=i12)
        bp_sb = consts.tile([BSH, C], F32)
        nc.scalar.dma_start(
            out=bp_sb,
            in_=bass.AP(tensor=bp.tensor, offset=0, ap=[[0, BSH], [1, C]]),
        )
        qT_sb = consts.tile([128, NCHUNK, BSH], F32)
        # written by a casting tensor_copy from f32 PSUM, read by phase-A matmul
        mT_sb = consts.tile([128, NCHUNK, BSH, H], A_DT)

        # ---- P0a: qT[c_out, b] = wq @ (SCALE * x0^T), contraction over c_in ----
        for co in range(NCHUNK):
            ps_q = ps_misc.tile([128, BSH], F32, tag="misc")
            for ci in range(NCHUNK):
                nc.tensor.matmul(
                    ps_q,
                    lhsT=wqT_sb[:, ci, co * 128:(co + 1) * 128],
                    rhs=x0T_sb[:, ci, :],
                    start=(ci == 0), stop=(ci == NCHUNK - 1),
                )
            nc.vector.tensor_copy(qT_sb[:, co, :], ps_q)

        # ---- P0b: mT[c, b, h] = Wk_h^T @ qT_h  (contraction over d=64) ----
        for ci in range(NCHUNK):
            for h in range(H):
                po = (h % 2) * 64
                ch = h // 2
                ps_m = ps_misc.tile([128, BSH], F32, tag="misc")
                nc.tensor.matmul(
                    ps_m,
                    lhsT=wk_sb[po:po + 64, ch, ci * 128:(ci + 1) * 128],
                    rhs=qT_sb[po:po + 64, ch, :],
                    start=True, stop=True,
                )
                nc.vector.tensor_copy(mT_sb[:, ci, :, h], ps_m)

        ut_all = consts.tile([128, NCHUNK, BSH, H], F32)  # U^T[c, b, h]
        if dbg:
            nc.sync.dma_start(out=dbg["qT"], in_=qT_sb)
            nc.sync.dma_start(out=dbg["mT"], in_=mT_sb)

        # ---- per-batch main loop ----
        for b in range(BSH):
            # phase A: scores[h, n] = sum_c mT[c, h] * xT[c, n]; exp is fused
            # into the PSUM->SBUF move (no max subtraction needed: |scores|<8)
            attn = attn_pool.tile([H, N], F32, tag="attn")
            partials = small.tile([H, N // 512], F32, tag="partials")
            for nt in range(N // 512):
                xa = []
                for ci in range(NCHUNK):
                    t = xa_pool.tile([128, 512], A_DT, tag="xa")
                    nc.sync.dma_start(
                        out=t,
                        in_=xT[b, ci * 128:(ci + 1) * 128, nt * 512:(nt + 1) * 512],
                    )
                    xa.append(t)
                ps = ps_a.tile([H, 512], F32, tag="psA")
                for ci in range(NCHUNK):
                    nc.tensor.matmul(
                        ps,
                        lhsT=mT_sb[:, ci, b, :],
                        rhs=xa[ci],
                        start=(ci == 0), stop=(ci == NCHUNK - 1),
                    )
                nc.scalar.activation(
                    out=attn[:, nt * 512:(nt + 1) * 512], in_=ps,
                    func=mybir.ActivationFunctionType.Exp,
                    accum_out=partials[:, nt:nt + 1],
                )

            sums = small.tile([H, 1], F32, tag="sums")
            nc.vector.reduce_sum(sums, partials, axis=mybir.AxisListType.X)
            rsum = small.tile([H, 1], F32, tag="rsum")
            nc.vector.reciprocal(rsum, sums)

            # transpose attn -> attnT[n, h] chunks (PE transpose via identity);
            # the PSUM->SBUF copy also casts to the phase-C matmul dtype
            attnT = attn_pool.tile([128, N // 128, H], C_DT, tag="attnT")
            for nn in range(N // 128):
                ps_t = ps_a.tile([128, H], F32, tag="psAT")
                nc.tensor.transpose(
                    ps_t, in_=attn[:, nn * 128:(nn + 1) * 128], identity=i12_sb)
                nc.vector.tensor_copy(attnT[:, nn, :], ps_t)
            if dbg and b == 0:
                nc.sync.dma_start(out=dbg["attnT"], in_=attnT)

            # phase C: U[h, c] = sum_n attnT[n, h] * x[n, c]
            psU0 = ps_c.tile([H, 384], F32, tag="psC0")
            psU1 = ps_c.tile([H, 384], F32, tag="psC1")
            psU = [psU0, psU1]
            for nn in range(N // 128):
                xc = xc_pool.tile([128, C], C_DT, tag="xc")
                # issue phase-C loads on the other HWDGE engine so the two
                # x streams ride independent DMA queues
                nc.scalar.dma_start(out=xc, in_=x[b, nn * 128:(nn + 1) * 128, :])
                for j in range(2):
                    nc.tensor.matmul(
                        psU[j],
                        lhsT=attnT[:, nn, :],
                        rhs=xc[:, j * 384:(j + 1) * 384],
                        start=(nn == 0), stop=(nn == N // 128 - 1),
                    )
            # normalize by softmax sum while moving PSUM -> SBUF
            U_sb = small.tile([H, C], F32, tag="U")
            for j in range(2):
                nc.vector.tensor_scalar_mul(
                    out=U_sb[:, j * 384:(j + 1) * 384], in0=psU[j], scalar1=rsum,
                )

            if dbg and b == 0:
                nc.sync.dma_start(out=dbg["U"], in_=U_sb)
            # transpose U -> UT[c, h] chunks for the output projections
            for k in range(NCHUNK):
                ps_t = ps_misc.tile([128, H], F32, tag="misc")
                nc.tensor.transpose(ps_t, in_=U_sb[:, k * 128:(k + 1) * 128],
                                    identity=i12_sb)
                nc.vector.tensor_copy(ut_all[:, k, b, :], ps_t)

        # ---- P4a: ypre[h*64+d, b] = sum_c wvT[c, h*64+d] * UT[c, b, h] ----
        # these ride the sync queue, which is idle after the last xa tile
        wvT_sb = consts.tile([128, NCHUNK, C], F32, tag="wvT_sb")
        nc.sync.dma_start(out=wvT_sb, in_=wvT.rearrange("(a p) o -> p a o", p=128))
        wpT_sb = consts.tile([128, NCHUNK, C], F32, tag="wpT_sb")
        nc.sync.dma_start(out=wpT_sb, in_=wpT.rearrange("(a p) o -> p a o", p=128))
        ypT_sb = consts.tile([128, NCHUNK, BSH], F32)
        for h in range(H):
            ps_yp = ps_misc.tile([64, BSH], F32, tag="misc")
            for k in range(NCHUNK):
                nc.tensor.matmul(
                    ps_yp,
                    lhsT=wvT_sb[:, k, h * 64:(h + 1) * 64],
                    rhs=ut_all[:, k, :, h],
                    start=(k == 0), stop=(k == NCHUNK - 1),
                )
            po = (h % 2) * 64
            nc.vector.tensor_copy(ypT_sb[po:po + 64, h // 2, :], ps_yp)

        # ---- P4b: y[b, c_out] = sum_c ypT[c, b] * wpT[c, c_out] + bp ----
        y_sb = small.tile([BSH, C], F32, tag="y")
        for j in range(2):
            ps_y = ps_misc.tile([BSH, 384], F32, tag="misc")
            for k in range(NCHUNK):
                nc.tensor.matmul(
                    ps_y,
                    lhsT=ypT_sb[:, k, :],
                    rhs=wpT_sb[:, k, j * 384:(j + 1) * 384],
                    start=(k == 0), stop=(k == NCHUNK - 1),
                )
            nc.vector.tensor_add(
                out=y_sb[:, j * 384:(j + 1) * 384],
                in0=ps_y,
                in1=bp_sb[:, j * 384:(j + 1) * 384],
            )
        nc.sync.dma_start(out=y, in_=y_sb)


_CACHE = {}


def kernel(x, wq, wk, wv, wp, bp, trace=False):
    x = np.ascontiguousarray(x, dtype=np.float32)
    wq = np.asarray(wq, dtype=np.float32)
    wk = np.asarray(wk, dtype=np.float32)
    wv = np.asarray(wv, dtype=np.float32)
    wp = np.asarray(wp, dtype=np.float32)
    bp = np.asarray(bp, dtype=np.float32)

    if "nc" not in _CACHE:
        _CACHE["nc"] = build_kernel()
    nc = _CACHE["nc"]

    x_sh = x.reshape(NCORES, BSH, N, C)
    wqT = np.ascontiguousarray(wq.T)
    wkn = np.ascontiguousarray(wk)
    wvT = np.ascontiguousarray(wv.T)
    wpT = np.ascontiguousarray(wp.T)
    bp2 = np.ascontiguousarray(bp.reshape(1, C))
    i12 = np.eye(H, dtype=np.float32)

    in_maps = []
    for k in range(NCORES):
        xs = x_sh[k]
        in_maps.append({
            "xT": np.ascontiguousarray(xs.transpose(0, 2, 1)),
            "x": np.ascontiguousarray(xs),
            "x0T": np.ascontiguousarray((xs[:, 0, :] * SCALE).T),
            "wqT": wqT,
            "wk": wkn,
            "wvT": wvT,
            "wpT": wpT,
            "bp": bp2,
            "i12": i12,
        })

    res = run_bass_kernel_spmd(nc, in_maps, core_ids=list(range(NCORES)),
                               trace=trace)
    out = np.concatenate([res.results[k]["y"] for k in range(NCORES)], axis=0)
    out = out.reshape(B, 1, C)
    if trace:
        _CACHE["last_exec_time_ns"] = res.exec_time_ns
        _CACHE["last_results"] = res
    return out



# revision 31
# speedup vs baseline: 2.2172x; 2.2172x over previous
"""Trainium2 Bass kernel for nn_CrossAttention (single-CLS-query cross attention).

Reference computes, per batch b:
    q = x[b,0,:] @ wq.T                  (single CLS query)
    k = x[b] @ wk.T ; v = x[b] @ wv.T
    out = softmax(q k^T / sqrt(d)) v ; y = out @ wp.T + bp

With a single query the K/V projections fold away algebraically:
    scores[b,h,n] = M[b,h,:] . x[b,n,:]   with  M[b,h,:] = (SCALE*q_h) @ Wk_h
    U[b,h,:]      = sum_n exp(scores[b,h,n]) x[b,n,:]       (unnormalized)
    y[b]          = concat_h((U[b,h,:]/Z[b,h]) @ Wv_h.T) @ wp.T + bp

This version streams x through SBUF exactly ONCE per core (bf16), in the
natural [n, c] layout:
  - per 128-row n-tile, the 6 [128,128] sub-tiles are PE-transposed into
    PSUM (bf16) and copied to SBUF (copies spread over DVE/Act/Pool);
  - scores^T[n,h]: 6 weight-stationary matmuls (lhsT = transposed x tile,
    rhs = mT[c,12]) accumulate into PSUM — only 12 moving columns each;
  - exp fused in the PSUM->SBUF activation (|scores| small, no max sub);
  - softmax denominators via a ones-vector matmul (lhsT = attnT);
  - U^T[c,h]: 6 weight-stationary matmuls (lhsT = native x tile,
    rhs = attnT[n,12]) accumulate over all n-tiles.
Normalization by 1/Z is applied at the tiny per-head projection stage via a
broadcast matrix built with an indicator matmul (rsumE).

Distribution: pure data parallel over batch B=32 across 8 cores (4 per
core), no collectives.
"""

import numpy as np
import ml_dtypes

import concourse.bass as bass
import concourse.tile as tile
from concourse import bacc, mybir
from concourse.bass_utils import run_bass_kernel_spmd

# Problem constants (hardcoded per the harness contract).
B, N, C = 32, 4096, 768
H, D = 12, 64
SCALE = D ** -0.5
NCORES = 8
BSH = B // NCORES          # batches per core
NCHUNK = C // 128          # 6
GDMA = 8                   # n-tiles per x DMA chunk / processing group
NT = N // 128              # 32 n-tiles per batch
NCH = NT // GDMA           # 4 dma chunks (= groups) per batch
GP = GDMA                  # n-tiles per processing group (full PSUM bank)

F32 = mybir.dt.float32
BF16 = mybir.dt.bfloat16


def build_kernel():
    nc = bacc.Bacc("TRN2", target_bir_lowering=False, debug=False,
                   num_devices=NCORES)

    x = nc.dram_tensor("x", [BSH, N, C], BF16, kind="ExternalInput")
    x0T = nc.dram_tensor("x0T", [C, BSH], BF16, kind="ExternalInput")
    wqT = nc.dram_tensor("wqT", [C, C], BF16, kind="ExternalInput")
    wk = nc.dram_tensor("wk", [C, C], BF16, kind="ExternalInput")
    wvT = nc.dram_tensor("wvT", [C, C], BF16, kind="ExternalInput")
    wpT = nc.dram_tensor("wpT", [C, C], BF16, kind="ExternalInput")
    i128 = nc.dram_tensor("i128", [128, 128], BF16, kind="ExternalInput")
    ones = nc.dram_tensor("ones", [128, 1], BF16, kind="ExternalInput")
    e6 = nc.dram_tensor("e6", [H, NCHUNK, 128], F32, kind="ExternalInput")
    bpT = nc.dram_tensor("bpT", [128, NCHUNK, 1], F32, kind="ExternalInput")
    yT = nc.dram_tensor("yT", [128, NCHUNK, BSH], F32, kind="ExternalOutput")

    with tile.TileContext(nc) as tc:
        cross_attn_kernel(tc, yT.ap(), x.ap(), x0T.ap(), wqT.ap(), wk.ap(),
                          wvT.ap(), wpT.ap(), i128.ap(), ones.ap(), e6.ap(),
                          bpT.ap())
    nc.compile()
    return nc


def cross_attn_kernel(tc, yT, x, x0T, wqT, wk, wvT, wpT, i128, ones, e6, bpT):
    from contextlib import ExitStack
    ctx = ExitStack()
    nc = tc.nc
    with ctx:
        consts = ctx.enter_context(tc.tile_pool(name="consts", bufs=1))
        xc_pool = ctx.enter_context(tc.tile_pool(name="xc", bufs=2))
        xt_pool = ctx.enter_context(tc.tile_pool(name="xt", bufs=2))
        attn_pool = ctx.enter_context(tc.tile_pool(name="attn", bufs=3))
        small = ctx.enter_context(tc.tile_pool(name="small", bufs=2))
        ps_pack = ctx.enter_context(tc.tile_pool(name="ps_pack", bufs=2,
                                                 space="PSUM"))
        ps_s = ctx.enter_context(tc.tile_pool(name="ps_s", bufs=2,
                                              space="PSUM"))
        ps_u = ctx.enter_context(tc.tile_pool(name="ps_u", bufs=2,
                                              space="PSUM"))
        ps_misc = ctx.enter_context(tc.tile_pool(name="ps_misc", bufs=1,
                                                 space="PSUM"))

        # ---- constant loads (scalar HWDGE queue; sync queue streams x) ----
        def load_w(ap_dram, name):
            t = consts.tile([128, NCHUNK, C], BF16, tag=name)
            nc.scalar.dma_start(out=t,
                                in_=ap_dram.rearrange("(a p) o -> p a o", p=128))
            return t

        wqT_sb = load_w(wqT, "wqT_sb")
        wk_sb = load_w(wk, "wk_sb")
        x0T_sb = consts.tile([128, NCHUNK, BSH], BF16)
        nc.scalar.dma_start(out=x0T_sb,
                            in_=x0T.rearrange("(a p) b -> p a b", p=128))
        i128_sb = consts.tile([128, 128], BF16)
        nc.scalar.dma_start(out=i128_sb, in_=i128)
        ones_sb = consts.tile([128, 1], BF16)
        nc.scalar.dma_start(out=ones_sb, in_=ones)
        e6_sb = consts.tile([H, NCHUNK, 128], F32)
        nc.scalar.dma_start(out=e6_sb, in_=e6)
        bpT_sb = consts.tile([128, NCHUNK, 1], F32)
        nc.scalar.dma_start(out=bpT_sb, in_=bpT)

        qT_sb = consts.tile([128, NCHUNK, BSH], BF16)
        mT_sb = consts.tile([128, NCHUNK, BSH, H], BF16)
        ut_sb = consts.tile([128, NCHUNK, H, BSH], BF16)
        sums_sb = consts.tile([H, BSH], F32)

        # ---- P0a: qT[c_out, b] = wq @ (SCALE * x0^T) ----
        for co in range(NCHUNK):
            ps_q = ps_misc.tile([128, BSH], F32, tag="misc")
            for ci in range(NCHUNK):
                nc.tensor.matmul(
                    ps_q,
                    lhsT=wqT_sb[:, ci, co * 128:(co + 1) * 128],
                    rhs=x0T_sb[:, ci, :],
                    start=(ci == 0), stop=(ci == NCHUNK - 1),
                )
            nc.vector.tensor_copy(qT_sb[:, co, :], ps_q)

        # ---- P0b: mT[c, b, h] = Wk_h^T @ qT_h  (contraction over d=64) ----
        # whole-tile single-shot groups: sliced psum groups with varying
        # weights miscompute on HW, so one tile per head. Cycle through the
        # (still idle) main-loop psum rings to deepen the mm->copy pipeline.
        p0_slots = [(ps_misc, "misc"), (ps_s, "s"), (ps_u, "psU"),
                    (ps_s, "s"), (ps_u, "psU"), (ps_misc, "sums")]
        for ci in range(NCHUNK):
            for h in range(H):
                po = (h % 2) * 64
                ch = h // 2
                pool, ptag = p0_slots[(ci * H + h) % len(p0_slots)]
                ps_m = pool.tile([128, BSH], F32, tag=ptag)
                nc.tensor.matmul(
                    ps_m,
                    lhsT=wk_sb[po:po + 64, ch, ci * 128:(ci + 1) * 128],
                    rhs=qT_sb[po:po + 64, ch, :],
                    start=True, stop=True,
                )
                eng = nc.vector if h % 2 == 0 else nc.scalar
                if eng is nc.scalar:
                    nc.scalar.activation(
                        out=mT_sb[:, ci, :, h], in_=ps_m,
                        func=mybir.ActivationFunctionType.Copy)
                else:
                    nc.vector.tensor_copy(mT_sb[:, ci, :, h], ps_m)

        # ---- main loop: one streaming pass over x ----
        # copy-engine round-robin for the transposed-tile PSUM->SBUF drains
        copy_engines = [nc.vector, nc.scalar, nc.vector,
                        nc.vector, nc.scalar, nc.vector]
        ce_idx = 0

        u_acc = consts.tile([128, NCHUNK, H], F32)
        for b in range(BSH):
            ps_sum = ps_misc.tile([H, 1], F32, tag="sums")
            xcs = {}
            ats = {}
            for t in range(NCH):
                xc = xc_pool.tile([128, GDMA, C], BF16, tag=f"xc{t % 2}")
                xcs[t] = xc
                nc.sync.dma_start(
                    out=xc,
                    in_=x[b, t * GDMA * 128:(t + 1) * GDMA * 128, :]
                    .rearrange("(a p) c -> p a c", p=128),
                )
                # transpose the 6x8 [128,128] subtiles of this chunk
                # (disjoint bytes within the bank; verified exact on HW)
                xt = []
                for ci in range(NCHUNK):
                    pk = ps_pack.tile([128, GP, 128], BF16, tag="pack")
                    for jj in range(GP):
                        nc.tensor.matmul(
                            pk[:, jj, :],
                            lhsT=xc[:, jj, ci * 128:(ci + 1) * 128],
                            rhs=i128_sb,
                            is_transpose=True,
                            start=(jj == 0), stop=(jj == GP - 1),
                        )
                    st = xt_pool.tile([128, GP, 128], BF16, tag=f"xt{ci}")
                    eng = copy_engines[ce_idx % len(copy_engines)]
                    ce_idx += 1
                    if eng is nc.scalar:
                        nc.scalar.activation(
                            out=st, in_=pk,
                            func=mybir.ActivationFunctionType.Copy)
                    else:
                        eng.tensor_copy(st, pk)
                    xt.append(st)

                # scores^T: whole-tile accumulation group per n-tile
                # (sliced psum groups with varying weights miscompute on HW)
                attnT = attn_pool.tile([128, GP, H], BF16, tag="attnT")
                ats[t] = attnT
                for jj in range(GP):
                    pss = ps_s.tile([128, H], F32, tag="s")
                    for ci in range(NCHUNK):
                        nc.tensor.matmul(
                            pss,
                            lhsT=xt[ci][:, jj, :],
                            rhs=mT_sb[:, ci, b, :],
                            start=(ci == 0), stop=(ci == NCHUNK - 1),
                        )
                    nc.scalar.activation(
                        out=attnT[:, jj, :], in_=pss,
                        func=mybir.ActivationFunctionType.Exp)

                # softmax denominators: whole-tile per-batch group
                for jj in range(GP):
                    nc.tensor.matmul(
                        ps_sum,
                        lhsT=attnT[:, jj, :],
                        rhs=ones_sb,
                        start=(t == 0 and jj == 0),
                        stop=(t == NCH - 1 and jj == GP - 1),
                    )

                # U^T[c, h]: whole-tile groups per ci spanning a chunk
                # PAIR (x tiles stationary), drained into the SBUF
                # accumulator after each odd chunk
                if t % 2 == 1:
                    for ci in range(NCHUNK):
                        psUc = ps_u.tile([128, H], F32, tag="psU")
                        for tt in (t - 1, t):
                            for jj in range(GP):
                                nc.tensor.matmul(
                                    psUc,
                                    lhsT=xcs[tt][:, jj,
                                                 ci * 128:(ci + 1) * 128],
                                    rhs=ats[tt][:, jj, :],
                                    start=(tt == t - 1 and jj == 0),
                                    stop=(tt == t and jj == GP - 1),
                                )
                        if t == 1:
                            nc.vector.tensor_copy(u_acc[:, ci, :], psUc)
                        else:
                            nc.vector.tensor_add(ut_sb[:, ci, :, b], psUc,
                                                 u_acc[:, ci, :])
            # drain this batch's softmax sums
            nc.vector.tensor_copy(sums_sb[:, b:b + 1], ps_sum)

        # ---- normalization: rsumE[h*64+d (by chunk), b] = 1 / Z[h, b] ----
        rsum_sb = small.tile([H, BSH], F32, tag="rsum")
        nc.vector.reciprocal(rsum_sb, sums_sb)
        rsumE_sb = consts.tile([128, NCHUNK, BSH], F32)
        for a in range(NCHUNK):
            ps_e = ps_misc.tile([128, BSH], F32, tag="misc")
            nc.tensor.matmul(ps_e, lhsT=e6_sb[:, a, :], rhs=rsum_sb,
                             start=True, stop=True)
            nc.vector.tensor_copy(rsumE_sb[:, a, :], ps_e)

        # ---- P4a: ypre[h*64+d, b] = sum_c wvT[c, h*64+d] UT[c, b, h] ----
        wvT_sb = load_w(wvT, "wvT_sb")
        wpT_sb = load_w(wpT, "wpT_sb")
        ypT_sb = consts.tile([128, NCHUNK, BSH], BF16)
        for h in range(H):
            ps_yp = ps_misc.tile([64, BSH], F32, tag="misc")
            for k in range(NCHUNK):
                nc.tensor.matmul(
                    ps_yp,
                    lhsT=wvT_sb[:, k, h * 64:(h + 1) * 64],
                    rhs=ut_sb[:, k, h, :],
                    start=(k == 0), stop=(k == NCHUNK - 1),
                )
            po = (h % 2) * 64
            ch = h // 2
            nc.vector.tensor_mul(
                ypT_sb[po:po + 64, ch, :],
                ps_yp,
                rsumE_sb[po:po + 64, ch, :],
            )

        # ---- P4b: yT[c_out, b] = wpT^T @ ypre + bp ----
        for co in range(NCHUNK):
            ps_y = ps_misc.tile([128, BSH], F32, tag="misc")
            for k in range(NCHUNK):
                nc.tensor.matmul(
                    ps_y,
                    lhsT=wpT_sb[:, k, co * 128:(co + 1) * 128],
                    rhs=ypT_sb[:, k, :],
                    start=(k == 0), stop=(k == NCHUNK - 1),
                )
            yt_sb = small.tile([128, BSH], F32, tag=f"y{co}")
            nc.vector.tensor_scalar_add(
                out=yt_sb, in0=ps_y, scalar1=bpT_sb[:, co, :],
            )
            nc.sync.dma_start(out=yT[:, co, :], in_=yt_sb)


_CACHE = {}


def kernel(x, wq, wk, wv, wp, bp, trace=False):
    x = np.asarray(x, dtype=np.float32)
    wq = np.asarray(wq, dtype=np.float32)
    wk = np.asarray(wk, dtype=np.float32)
    wv = np.asarray(wv, dtype=np.float32)
    wp = np.asarray(wp, dtype=np.float32)
    bp = np.asarray(bp, dtype=np.float32)

    if "nc" not in _CACHE:
        _CACHE["nc"] = build_kernel()
    nc = _CACHE["nc"]

    bf = ml_dtypes.bfloat16
    x_sh = x.reshape(NCORES, BSH, N, C)
    wqT = np.ascontiguousarray(wq.T).astype(bf)
    wkn = np.ascontiguousarray(wk).astype(bf)
    wvT = np.ascontiguousarray(wv.T).astype(bf)
    wpT = np.ascontiguousarray(wp.T).astype(bf)
    i128 = np.eye(128, dtype=bf)
    ones = np.ones((128, 1), dtype=bf)
    # e6[h, a, p] = 1 iff head of channel a*128+p is h
    ch_idx = (np.arange(C) // D)  # head of each channel
    e6 = np.zeros((H, NCHUNK, 128), dtype=np.float32)
    e6[ch_idx, np.arange(C) // 128, np.arange(C) % 128] = 1.0
    bpT = np.ascontiguousarray(
        bp.reshape(NCHUNK, 128, 1).transpose(1, 0, 2)).astype(np.float32)

    in_maps = []
    for k in range(NCORES):
        xs = x_sh[k]
        in_maps.append({
            "x": np.ascontiguousarray(xs).astype(bf),
            "x0T": np.ascontiguousarray((xs[:, 0, :] * SCALE).T).astype(bf),
            "wqT": wqT,
            "wk": wkn,
            "wvT": wvT,
            "wpT": wpT,
            "i128": i128,
            "ones": ones,
            "e6": e6,
            "bpT": bpT,
        })

    res = run_bass_kernel_spmd(nc, in_maps, core_ids=list(range(NCORES)),
                               trace=trace)
    # yT[p, k, b] -> y[b, k*128+p]
    out = np.concatenate(
        [np.asarray(res.results[k]["yT"]).transpose(2, 1, 0).reshape(BSH, C)
         for k in range(NCORES)], axis=0)
    out = np.ascontiguousarray(out.reshape(B, 1, C), dtype=np.float32)
    if trace:
        _CACHE["last_exec_time_ns"] = res.exec_time_ns
        _CACHE["last_results"] = res
    return out
